# revision 13
# baseline (speedup 1.0000x reference)
"""Trainium2 Bass kernel for nn_MultiHeadMixer.

Reference computation (B=4, S=2048, E=1024, H=16, D=64):
    xp = x @ inp_w.T + inp_b                      # (B,S,E)
    xh[b,h,d,s] = xp[b,s,h*D+d]
    y0[b,h,d,t] = sum_{s<=t} xh[b,h,d,s] * weight[h,t-s]   # causal Toeplitz
    y2 = y0 / cumsum(weight)[h,t] + bias[h,t]
    out[b,t,:] = (y2 reshaped to (E,)) @ out_w.T

Sharding (8 cores): core c = (batch-pair p=c//4, head-group g=c%4).
Each core computes batches {2p, 2p+1} for heads {4g..4g+3} (feature cols
F=[256g,256g+256)) and emits a full-(E) bf16 partial of out[b].T per
batch; host sums the four head-group partials per batch and transposes.

On-device phases (PE at full 128 width everywhere):
  proj1:  xp[s, 256b+f] = sum_e xT_b[e,s] w1[e,f]           (K=e)
  mixer:  block-T: y0T[tau,(b,d)] += Tblk[sig,tau]^T xp_i[sig,(b,d)]
          with Tblk = strip[:, 128*(j-i):+128]; both batches of one head
          fill the 128-wide moving tensor; the Delta=j-i loop reuses one
          stationary across a t-quad.
  epi:    y2T = y0T*invn_col + bias_col (per-PARTITION [128,1] scalars)
  transp: PE transposes y2T 64 cols/batch into head-pair PSUM banks.
  proj2:  outT_b[e',t] = sum_c w2[c,e'] y2_b[c,t]           (K=c)

Schedule: the PE stream pipelines one quad deep (transposes/proj2 of
quad q run between mixer segments of q+1) so cross-engine epilogue and
copy latencies never stall the PE.  Loads ride the SP HWDGE ring, output
stores the Activation ring.

PSUM: every bank's first matmul after reuse is start=True (clears the
2KiB zero-region has_written bits); later first-touches overwrite and
repeat-touches accumulate.
"""

import contextlib

import numpy as np
import ml_dtypes

import concourse.bass as bass
import concourse.bacc as bacc
import concourse.mybir as mybir
import concourse.tile as tile
from concourse.bass_utils import run_bass_kernel_spmd

B, S, E, H = 4, 2048, 1024, 16
D = E // H
N_CORES = 8
HPC = 4          # heads per core
FPC = 256        # feature cols per core
SB = S // 128    # 16 s/t-blocks
TQ = S // 512    # 4 t-quads

BF16 = mybir.dt.bfloat16
F32 = mybir.dt.float32
NPBF16 = ml_dtypes.bfloat16

_CACHED = {}

POOL_SPECS = [
    ("xtt", 8, None), ("tpc", 1, None), ("wc", 2, None), ("cst", 3, None),
    ("xp", SB, None), ("y2t", 26, None), ("y2s", 8, None), ("ost", 16, None),
    ("psAP", 2, "PSUM"), ("psm", 4, "PSUM"), ("psT", 2, "PSUM"),
]


def emit_loads(nc, pools, aps, pfx=""):
    xtt_d, w1c_d, b1x_d, tpc_d, ivb_d, w2c_d, ident_d, outX = aps
    t = {}
    t["w1c"] = pools["wc"].tile([128, 2048], BF16, tag="wc", name=f"{pfx}w1c")
    nc.sync.dma_start(t["w1c"][:], w1c_d[:])
    for mg in range(4):
        for b in range(2):
            x = pools["xtt"].tile([128, 4096], BF16, tag="xtt",
                                  name=f"{pfx}xtt_{b}_{mg}")
            eng = nc.sync if b == 0 else nc.scalar
            eng.dma_start(x[:], xtt_d[b][mg])
            t[("xtt", b, mg)] = x
    t["b1"] = pools["cst"].tile([128, 2 * FPC], F32, tag="cst", name=f"{pfx}b1_t")
    nc.sync.dma_start(t["b1"][:], b1x_d[:])
    t["tpc"] = pools["tpc"].tile([128, 4 * 2048], BF16, tag="tpc", name=f"{pfx}tpc")
    nc.scalar.dma_start(t["tpc"][:], tpc_d[:])
    t["ivb"] = pools["cst"].tile([128, 128], F32, tag="cst", name=f"{pfx}ivb")
    nc.sync.dma_start(t["ivb"][:], ivb_d[:])
    t["ident"] = pools["cst"].tile([128, 128], BF16, tag="cst", name=f"{pfx}ident")
    nc.sync.dma_start(t["ident"][:], ident_d[:])
    t["w2c"] = pools["wc"].tile([128, 2048], BF16, tag="wc", name=f"{pfx}w2c")
    nc.scalar.dma_start(t["w2c"][:], w2c_d[:])
    return t


def emit_compute(nc, pools, t, aps, pfx=""):
    xtt_d, w1c_d, b1x_d, tpc_d, ivb_d, w2c_d, ident_d, outX = aps
    AF = mybir.ActivationFunctionType
    ALU = mybir.AluOpType
    w1c, tpc, ivb, ident, w2c = (t["w1c"], t["tpc"], t["ivb"], t["ident"],
                                 t["w2c"])

    def invn_col(h, j):
        return ivb[:, 16 * h + j:16 * h + j + 1]

    def bias_col(h, j):
        return ivb[:, 64 + 16 * h + j:64 + 16 * h + j + 1]

    # ---- proj1 ----
    xp_t = []
    for m in range(SB):
        mg, mm = m // 4, m % 4
        ps = pools["psAP"].tile([128, 512], F32, tag="psAP", name=f"{pfx}ps1_{m}")
        for b in range(2):
            for k in range(8):
                nc.tensor.matmul(
                    ps[:, FPC * b:FPC * (b + 1)],
                    t[("xtt", b, mg)][:, 1024 * mm + 128 * k:
                                      1024 * mm + 128 * (k + 1)],
                    w1c[:, FPC * k:FPC * (k + 1)],
                    start=(b == 0 and k == 0),
                    stop=(b == 1 and k == 7),
                    skip_group_check=True,
                )
        xp = pools["xp"].tile([128, 512], BF16, tag="xp", name=f"{pfx}xp_{m}")
        nc.vector.tensor_add(xp[:], ps[:], t["b1"][:])
        xp_t.append(xp)

    def moving(i, h):
        a = xp_t[i][:].rearrange("p (b h d) -> p b h d", b=2, h=HPC)
        return a[:, :, h, :]

    y2t = {}
    y2s = {}

    def mix(q, hpair):
        for h in (2 * hpair, 2 * hpair + 1):
            ps = pools["psm"].tile([128, 512], F32, tag="psm",
                                   name=f"{pfx}psm_{h}_{q}")
            for dlt in range(4 * q + 4):
                for j in range(max(4 * q, dlt), 4 * q + 4):
                    i = j - dlt
                    jj = j - 4 * q
                    nc.tensor.matmul(
                        ps[:, 128 * jj:128 * (jj + 1)],
                        tpc[:, 2048 * h + 128 * dlt:
                            2048 * h + 128 * (dlt + 1)],
                        moving(i, h),
                        start=(dlt == 0 and jj == 0),
                        stop=(dlt == 4 * q + 3 and jj == 3),
                        skip_group_check=True,
                    )
            for jj in range(4):
                j = 4 * q + jj
                y = pools["y2t"].tile([128, 128], BF16, tag="y2t",
                                      name=f"{pfx}y2t_{h}_{j}")
                if h % 2 == 0:
                    nc.scalar.activation(
                        y[:], ps[:, 128 * jj:128 * (jj + 1)],
                        AF.Identity,
                        bias=bias_col(h, j), scale=invn_col(h, j))
                else:
                    nc.vector.tensor_scalar(
                        y[:], ps[:, 128 * jj:128 * (jj + 1)],
                        invn_col(h, j), bias_col(h, j),
                        ALU.mult, ALU.add)
                y2t[(h, jj, q)] = y

    def transp(q, hp):
        for b in range(2):
            pst = pools["psT"].tile([128, 1024], BF16, tag="psT",
                                    name=f"{pfx}psT_{hp}_{b}_{q}")
            for hh in range(2):
                for jj in range(4):
                    nc.tensor.transpose(
                        pst[64 * hh:64 * (hh + 1),
                            128 * jj:128 * (jj + 1)],
                        y2t[(2 * hp + hh, jj, q)][:, 64 * b:64 * (b + 1)],
                        ident[:])
            ys = pools["y2s"].tile([128, 512], BF16, tag="y2s",
                                   name=f"{pfx}y2s_{b}_{hp}_{q}")
            if (hp + b) % 2 == 0:
                nc.vector.tensor_copy(ys[:], pst[:, 0:512])
            else:
                nc.scalar.copy(ys[:], pst[:, 0:512])
            y2s[(b, hp, q)] = ys

    ost_t = {}

    def proj2(q):
        for n in range(8):
            pso = {}
            for hp in range(2):
                for b in range(2):
                    if hp == 0:
                        pso[b] = pools["psAP"].tile(
                            [128, 512], F32, tag="psAP",
                            name=f"{pfx}ps2_{b}_{n}_{q}")
                    nc.tensor.matmul(
                        pso[b][:],
                        w2c[:, 1024 * hp + 128 * n:
                            1024 * hp + 128 * (n + 1)],
                        y2s[(b, hp, q)][:],
                        start=(hp == 0),
                        stop=(hp == 1),
                        skip_group_check=True,
                    )
            for b in range(2):
                if q == 0:
                    ost_t[(b, n)] = pools["ost"].tile(
                        [128, 2048], BF16, tag="ost", name=f"{pfx}ost_{b}_{n}")
                ost = ost_t[(b, n)]
                if n % 2 == 0:
                    nc.vector.tensor_copy(
                        ost[:, 512 * q:512 * (q + 1)], pso[b][:])
                else:
                    nc.scalar.copy(
                        ost[:, 512 * q:512 * (q + 1)], pso[b][:])
                if q == 3:
                    # stores ride the (otherwise idle) GpSimd SWDGE ring
                    nc.gpsimd.dma_start(
                        outX[b][128 * n:128 * (n + 1), :], ost[:])

    # ---- pipelined PE stream ----
    mix(0, 0)
    mix(0, 1)
    transp(0, 0)
    mix(1, 0)
    transp(0, 1)
    proj2(0)
    mix(1, 1)
    transp(1, 0)
    mix(2, 0)
    transp(1, 1)
    proj2(1)
    mix(2, 1)
    transp(2, 0)
    mix(3, 0)
    transp(2, 1)
    proj2(2)
    mix(3, 1)
    transp(3, 0)
    transp(3, 1)
    proj2(3)


def build_program(loop_n=None, mode="full"):
    nc = bacc.Bacc("TRN2", target_bir_lowering=False, debug=False,
                   num_devices=N_CORES)

    aps = (
        nc.dram_tensor("xtt", [2, 4, 128, 4096], BF16,
                       kind="ExternalInput").ap(),
        nc.dram_tensor("w1c", [128, 2048], BF16, kind="ExternalInput").ap(),
        nc.dram_tensor("b1x", [128, 2 * FPC], F32, kind="ExternalInput").ap(),
        nc.dram_tensor("tpc", [128, 4 * 2048], BF16,
                       kind="ExternalInput").ap(),
        nc.dram_tensor("ivb", [128, 128], F32, kind="ExternalInput").ap(),
        nc.dram_tensor("w2c", [128, 2048], BF16, kind="ExternalInput").ap(),
        nc.dram_tensor("ident", [128, 128], BF16, kind="ExternalInput").ap(),
        nc.dram_tensor("outX", [2, E, S], BF16, kind="ExternalOutput").ap(),
    )

    U = 8
    with tile.TileContext(nc) as tc, contextlib.ExitStack() as es:
        pools = {}
        for name, bufs, space in POOL_SPECS:
            kw = dict(space=space) if space else {}
            pools[name] = es.enter_context(
                tc.tile_pool(name=name, bufs=bufs, **kw))

        def body(pfx=""):
            t = emit_loads(nc, pools, aps, pfx)
            emit_compute(nc, pools, t, aps, pfx)

        if mode == "full":
            if not loop_n:
                body()
            else:
                main, rem = divmod(loop_n, U)
                if main:
                    with tc.For_i(0, main, 1, staggered_reset=True):
                        for u in range(U):
                            body(f"u{u}_")
                if rem:
                    with tc.For_i(0, rem, 1, staggered_reset=True):
                        body("r_")
        elif mode == "compute":
            t = emit_loads(nc, pools, aps)
            with (tc.For_i(0, loop_n, 1, staggered_reset=True)
                  if loop_n else contextlib.nullcontext()):
                emit_compute(nc, pools, t, aps)
        elif mode == "dma":
            with (tc.For_i(0, loop_n, 1, staggered_reset=True)
                  if loop_n else contextlib.nullcontext()):
                emit_loads(nc, pools, aps)
        else:
            raise ValueError(mode)

    nc.compile()
    return nc


def host_prep(x, weight, bias, inp_w, inp_b, out_w):
    """Build the 8 per-core input maps (host-side shard + layout prep)."""
    x = np.asarray(x, np.float32)
    weight = np.asarray(weight, np.float32)
    bias = np.asarray(bias, np.float32)
    inp_w = np.asarray(inp_w, np.float32)
    inp_b = np.asarray(inp_b, np.float32)
    out_w = np.asarray(out_w, np.float32)

    invn = 1.0 / np.cumsum(weight, axis=1)
    ident = np.eye(128, dtype=NPBF16)

    # xtt[b, mg][p, (mm,k,s)]: xT-block pretiling so proj1 tile m needs
    # only xtt[:, m//4]
    xtt_p = []
    for p in range(2):
        per_b = []
        for b in (2 * p, 2 * p + 1):
            A = np.ascontiguousarray(x[b].T)                 # [E, S]
            arr = A.reshape(8, 128, 16, 128).transpose(2, 1, 0, 3)  # [m,p,k,s]
            arr = arr.reshape(4, 4, 128, 8, 128).transpose(0, 2, 1, 3, 4)
            per_b.append(arr.reshape(4, 128, 4096))
        xtt_p.append(np.stack(per_b).astype(NPBF16))         # [2,4,128,4096]

    g_pack = []
    for g in range(4):
        cols = slice(FPC * g, FPC * (g + 1))
        w1c = (inp_w[cols, :].T.reshape(8, 128, FPC)
               .transpose(1, 0, 2).reshape(128, 2048)).astype(NPBF16)
        b1row = np.concatenate([inp_b[cols], inp_b[cols]])
        b1x = np.broadcast_to(b1row, (128, 2 * FPC)).astype(np.float32).copy()
        w2c = (out_w[:, cols].T.reshape(2, 128, E)
               .transpose(1, 0, 2).reshape(128, 2048)).astype(NPBF16)
        tpc = np.zeros((128, 4 * 2048), np.float32)
        ivb = np.zeros((128, 128), np.float32)
        for hl in range(HPC):
            hgl = 4 * g + hl
            wrow = weight[hgl]
            wpad = np.concatenate([np.zeros(127, np.float32), wrow])
            tpc[:, 2048 * hl:2048 * (hl + 1)] = np.lib.stride_tricks.as_strided(
                wpad[127:], shape=(128, S), strides=(-4, 4))
            ivb[:, 16 * hl:16 * (hl + 1)] = invn[hgl].reshape(SB, 128).T
            ivb[:, 64 + 16 * hl:64 + 16 * (hl + 1)] = \
                bias[hgl].reshape(SB, 128).T
        g_pack.append(dict(w1c=w1c, b1x=b1x, w2c=w2c,
                           tpc=tpc.astype(NPBF16), ivb=ivb, ident=ident))

    in_maps = []
    for c in range(N_CORES):
        p, g = c // 4, c % 4
        m = dict(g_pack[g])
        m["xtt"] = xtt_p[p]
        in_maps.append(m)
    return in_maps


def kernel(x, weight, bias, inp_w, inp_b, out_w):
    if "nc" not in _CACHED:
        _CACHED["nc"] = build_program()
    nc = _CACHED["nc"]

    in_maps = host_prep(x, weight, bias, inp_w, inp_b, out_w)
    res = run_bass_kernel_spmd(nc, in_maps, core_ids=list(range(N_CORES)))

    out = np.empty((B, S, E), np.float32)
    for b in range(B):
        p, bb = b // 2, b % 2
        acc = np.zeros((E, S), np.float32)
        for g in range(4):
            acc += np.asarray(res.results[4 * p + g]["outX"][bb],
                              dtype=np.float32)
        out[b] = acc.T
    return out


# revision 14
# speedup vs baseline: 2.1378x; 2.1378x over previous
"""Trainium2 Bass kernel for nn_MultiHeadMixer.

Reference computation (B=4, S=2048, E=1024, H=16, D=64):
    xp = x @ inp_w.T + inp_b                      # (B,S,E)
    xh[b,h,d,s] = xp[b,s,h*D+d]
    y0[b,h,d,t] = sum_{s<=t} xh[b,h,d,s] * weight[h,t-s]   # causal Toeplitz
    y2 = y0 / cumsum(weight)[h,t] + bias[h,t]
    out[b,t,:] = (y2 reshaped to (E,)) @ out_w.T

Sharding (8 cores): core c = (batch-pair p=c//4, head-group g=c%4).
Each core computes batches {2p, 2p+1} for heads {4g..4g+3} (feature cols
F=[256g,256g+256)) and emits a full-(E) bf16 partial of out[b].T per
batch; host sums the four head-group partials per batch and transposes.

On-device phases (PE at full 128 width everywhere):
  proj1:  xp[s, 256b+f] = sum_e xT_b[e,s] w1[e,f]           (K=e)
  mixer:  block-T: y0T[tau,(b,d)] += Tblk[sig,tau]^T xp_i[sig,(b,d)]
          with Tblk = strip[:, 128*(j-i):+128]; both batches of one head
          fill the 128-wide moving tensor; the Delta=j-i loop reuses one
          stationary across a t-quad.
  epi:    y2T = y0T*invn_col + bias_col (per-PARTITION [128,1] scalars)
  transp: PE transposes y2T 64 cols/batch into head-pair PSUM banks.
  proj2:  outT_b[e',t] = sum_c w2[c,e'] y2_b[c,t]           (K=c)

Schedule: the PE stream pipelines one quad deep (transposes/proj2 of
quad q run between mixer segments of q+1) so cross-engine epilogue and
copy latencies never stall the PE.  Loads ride the SP HWDGE ring, output
stores the Activation ring.

PSUM: every bank's first matmul after reuse is start=True (clears the
2KiB zero-region has_written bits); later first-touches overwrite and
repeat-touches accumulate.
"""

import contextlib

import numpy as np
import ml_dtypes

import concourse.bass as bass
import concourse.bacc as bacc
import concourse.mybir as mybir
import concourse.tile as tile
from concourse.bass_utils import run_bass_kernel_spmd

B, S, E, H = 4, 2048, 1024, 16
D = E // H
N_CORES = 8
HPC = 4          # heads per core
FPC = 256        # feature cols per core
SB = S // 128    # 16 s/t-blocks
TQ = S // 512    # 4 t-quads

BF16 = mybir.dt.bfloat16
F32 = mybir.dt.float32
NPBF16 = ml_dtypes.bfloat16

_CACHED = {}

POOL_SPECS = [
    ("xtt", 8, None), ("tpc", 1, None), ("wc", 2, None), ("cst", 3, None),
    ("xp", SB, None), ("y2t", 26, None), ("y2s", 8, None), ("ost", 16, None),
    ("psAP", 2, "PSUM"), ("psm", 4, "PSUM"), ("psT", 2, "PSUM"),
]


def emit_loads(nc, pools, aps, pfx=""):
    xtt_d, w1c_d, b1x_d, tpc_d, ivb_d, w2c_d, ident_d, outX = aps
    t = {}
    t["w1c"] = pools["wc"].tile([128, 2048], BF16, tag="wc", name=f"{pfx}w1c")
    nc.sync.dma_start(t["w1c"][:], w1c_d[:])
    for mg in range(4):
        for b in range(2):
            x = pools["xtt"].tile([128, 4096], BF16, tag="xtt",
                                  name=f"{pfx}xtt_{b}_{mg}")
            eng = nc.sync if b == 0 else nc.scalar
            eng.dma_start(x[:], xtt_d[b][mg])
            t[("xtt", b, mg)] = x
    t["b1"] = pools["cst"].tile([128, 2 * FPC], F32, tag="cst", name=f"{pfx}b1_t")
    nc.sync.dma_start(t["b1"][:], b1x_d[:])
    t["tpc"] = pools["tpc"].tile([128, 4 * 2048], BF16, tag="tpc", name=f"{pfx}tpc")
    nc.scalar.dma_start(t["tpc"][:], tpc_d[:])
    t["ivb"] = pools["cst"].tile([128, 128], F32, tag="cst", name=f"{pfx}ivb")
    nc.sync.dma_start(t["ivb"][:], ivb_d[:])
    t["ident"] = pools["cst"].tile([128, 128], BF16, tag="cst", name=f"{pfx}ident")
    nc.sync.dma_start(t["ident"][:], ident_d[:])
    t["w2c"] = pools["wc"].tile([128, 2048], BF16, tag="wc", name=f"{pfx}w2c")
    nc.scalar.dma_start(t["w2c"][:], w2c_d[:])
    return t


def emit_compute(nc, pools, t, aps, pfx=""):
    xtt_d, w1c_d, b1x_d, tpc_d, ivb_d, w2c_d, ident_d, outX = aps
    AF = mybir.ActivationFunctionType
    ALU = mybir.AluOpType
    w1c, tpc, ivb, ident, w2c = (t["w1c"], t["tpc"], t["ivb"], t["ident"],
                                 t["w2c"])

    def invn_col(h, j):
        return ivb[:, 16 * h + j:16 * h + j + 1]

    def bias_col(h, j):
        return ivb[:, 64 + 16 * h + j:64 + 16 * h + j + 1]

    # ---- proj1 ----
    xp_t = []
    for m in range(SB):
        mg, mm = m // 4, m % 4
        ps = pools["psAP"].tile([128, 512], F32, tag="psAP", name=f"{pfx}ps1_{m}")
        for b in range(2):
            for k in range(8):
                nc.tensor.matmul(
                    ps[:, FPC * b:FPC * (b + 1)],
                    t[("xtt", b, mg)][:, 1024 * mm + 128 * k:
                                      1024 * mm + 128 * (k + 1)],
                    w1c[:, FPC * k:FPC * (k + 1)],
                    start=(b == 0 and k == 0),
                    stop=(b == 1 and k == 7),
                    skip_group_check=True,
                )
        xp = pools["xp"].tile([128, 512], BF16, tag="xp", name=f"{pfx}xp_{m}")
        nc.vector.tensor_add(xp[:], ps[:], t["b1"][:])
        xp_t.append(xp)

    def moving(i, h):
        a = xp_t[i][:].rearrange("p (b h d) -> p b h d", b=2, h=HPC)
        return a[:, :, h, :]

    y2t = {}
    y2s = {}

    def mix(q, hpair):
        for h in (2 * hpair, 2 * hpair + 1):
            ps = pools["psm"].tile([128, 512], F32, tag="psm",
                                   name=f"{pfx}psm_{h}_{q}")
            for dlt in range(4 * q + 4):
                for j in range(max(4 * q, dlt), 4 * q + 4):
                    i = j - dlt
                    jj = j - 4 * q
                    nc.tensor.matmul(
                        ps[:, 128 * jj:128 * (jj + 1)],
                        tpc[:, 2048 * h + 128 * dlt:
                            2048 * h + 128 * (dlt + 1)],
                        moving(i, h),
                        start=(dlt == 0 and jj == 0),
                        stop=(dlt == 4 * q + 3 and jj == 3),
                        skip_group_check=True,
                    )
            for jj in range(4):
                j = 4 * q + jj
                y = pools["y2t"].tile([128, 128], BF16, tag="y2t",
                                      name=f"{pfx}y2t_{h}_{j}")
                if h % 2 == 0:
                    nc.scalar.activation(
                        y[:], ps[:, 128 * jj:128 * (jj + 1)],
                        AF.Identity,
                        bias=bias_col(h, j), scale=invn_col(h, j))
                else:
                    nc.vector.tensor_scalar(
                        y[:], ps[:, 128 * jj:128 * (jj + 1)],
                        invn_col(h, j), bias_col(h, j),
                        ALU.mult, ALU.add)
                y2t[(h, jj, q)] = y

    def transp(q, hp):
        for b in range(2):
            pst = pools["psT"].tile([128, 1024], BF16, tag="psT",
                                    name=f"{pfx}psT_{hp}_{b}_{q}")
            for hh in range(2):
                for jj in range(4):
                    nc.tensor.transpose(
                        pst[64 * hh:64 * (hh + 1),
                            128 * jj:128 * (jj + 1)],
                        y2t[(2 * hp + hh, jj, q)][:, 64 * b:64 * (b + 1)],
                        ident[:])
            ys = pools["y2s"].tile([128, 512], BF16, tag="y2s",
                                   name=f"{pfx}y2s_{b}_{hp}_{q}")
            if (hp + b) % 2 == 0:
                nc.vector.tensor_copy(ys[:], pst[:, 0:512])
            else:
                nc.scalar.copy(ys[:], pst[:, 0:512])
            y2s[(b, hp, q)] = ys

    ost_t = {}

    def proj2(q):
        for n in range(8):
            pso = {}
            for hp in range(2):
                for b in range(2):
                    if hp == 0:
                        pso[b] = pools["psAP"].tile(
                            [128, 512], F32, tag="psAP",
                            name=f"{pfx}ps2_{b}_{n}_{q}")
                    nc.tensor.matmul(
                        pso[b][:],
                        w2c[:, 1024 * hp + 128 * n:
                            1024 * hp + 128 * (n + 1)],
                        y2s[(b, hp, q)][:],
                        start=(hp == 0),
                        stop=(hp == 1),
                        skip_group_check=True,
                    )
            for b in range(2):
                if q == 0:
                    ost_t[(b, n)] = pools["ost"].tile(
                        [128, 2048], BF16, tag="ost", name=f"{pfx}ost_{b}_{n}")
                ost = ost_t[(b, n)]
                if n % 2 == 0:
                    nc.vector.tensor_copy(
                        ost[:, 512 * q:512 * (q + 1)], pso[b][:])
                else:
                    nc.scalar.copy(
                        ost[:, 512 * q:512 * (q + 1)], pso[b][:])
                if q == 3:
                    # stores ride the (otherwise idle) GpSimd SWDGE ring
                    nc.gpsimd.dma_start(
                        outX[b][128 * n:128 * (n + 1), :], ost[:])

    # ---- pipelined PE stream ----
    mix(0, 0)
    mix(0, 1)
    transp(0, 0)
    mix(1, 0)
    transp(0, 1)
    proj2(0)
    mix(1, 1)
    transp(1, 0)
    mix(2, 0)
    transp(1, 1)
    proj2(1)
    mix(2, 1)
    transp(2, 0)
    mix(3, 0)
    transp(2, 1)
    proj2(2)
    mix(3, 1)
    transp(3, 0)
    transp(3, 1)
    proj2(3)


def build_program(loop_n=None, mode="full"):
    nc = bacc.Bacc("TRN2", target_bir_lowering=False, debug=False,
                   num_devices=N_CORES)

    aps = (
        nc.dram_tensor("xtt", [2, 4, 128, 4096], BF16,
                       kind="ExternalInput").ap(),
        nc.dram_tensor("w1c", [128, 2048], BF16, kind="ExternalInput").ap(),
        nc.dram_tensor("b1x", [128, 2 * FPC], F32, kind="ExternalInput").ap(),
        nc.dram_tensor("tpc", [128, 4 * 2048], BF16,
                       kind="ExternalInput").ap(),
        nc.dram_tensor("ivb", [128, 128], F32, kind="ExternalInput").ap(),
        nc.dram_tensor("w2c", [128, 2048], BF16, kind="ExternalInput").ap(),
        nc.dram_tensor("ident", [128, 128], BF16, kind="ExternalInput").ap(),
        nc.dram_tensor("outX", [2, E, S], BF16, kind="ExternalOutput").ap(),
    )

    U = 1
    with tile.TileContext(nc) as tc, contextlib.ExitStack() as es:
        pools = {}
        for name, bufs, space in POOL_SPECS:
            kw = dict(space=space) if space else {}
            pools[name] = es.enter_context(
                tc.tile_pool(name=name, bufs=bufs, **kw))

        def body(pfx=""):
            t = emit_loads(nc, pools, aps, pfx)
            emit_compute(nc, pools, t, aps, pfx)

        if mode == "full":
            if not loop_n:
                body()
            else:
                main, rem = divmod(loop_n, U)
                if main:
                    with tc.For_i(0, main, 1, staggered_reset=True):
                        for u in range(U):
                            body(f"u{u}_")
                if rem:
                    with tc.For_i(0, rem, 1, staggered_reset=True):
                        body("r_")
        elif mode == "compute":
            t = emit_loads(nc, pools, aps)
            with (tc.For_i(0, loop_n, 1, staggered_reset=True)
                  if loop_n else contextlib.nullcontext()):
                emit_compute(nc, pools, t, aps)
        elif mode == "dma":
            with (tc.For_i(0, loop_n, 1, staggered_reset=True)
                  if loop_n else contextlib.nullcontext()):
                emit_loads(nc, pools, aps)
        else:
            raise ValueError(mode)

    nc.compile()
    return nc


def host_prep(x, weight, bias, inp_w, inp_b, out_w):
    """Build the 8 per-core input maps (host-side shard + layout prep)."""
    x = np.asarray(x, np.float32)
    weight = np.asarray(weight, np.float32)
    bias = np.asarray(bias, np.float32)
    inp_w = np.asarray(inp_w, np.float32)
    inp_b = np.asarray(inp_b, np.float32)
    out_w = np.asarray(out_w, np.float32)

    invn = 1.0 / np.cumsum(weight, axis=1)
    ident = np.eye(128, dtype=NPBF16)

    # xtt[b, mg][p, (mm,k,s)]: xT-block pretiling so proj1 tile m needs
    # only xtt[:, m//4]
    xtt_p = []
    for p in range(2):
        per_b = []
        for b in (2 * p, 2 * p + 1):
            A = np.ascontiguousarray(x[b].T)                 # [E, S]
            arr = A.reshape(8, 128, 16, 128).transpose(2, 1, 0, 3)  # [m,p,k,s]
            arr = arr.reshape(4, 4, 128, 8, 128).transpose(0, 2, 1, 3, 4)
            per_b.append(arr.reshape(4, 128, 4096))
        xtt_p.append(np.stack(per_b).astype(NPBF16))         # [2,4,128,4096]

    g_pack = []
    for g in range(4):
        cols = slice(FPC * g, FPC * (g + 1))
        w1c = (inp_w[cols, :].T.reshape(8, 128, FPC)
               .transpose(1, 0, 2).reshape(128, 2048)).astype(NPBF16)
        b1row = np.concatenate([inp_b[cols], inp_b[cols]])
        b1x = np.broadcast_to(b1row, (128, 2 * FPC)).astype(np.float32).copy()
        w2c = (out_w[:, cols].T.reshape(2, 128, E)
               .transpose(1, 0, 2).reshape(128, 2048)).astype(NPBF16)
        tpc = np.zeros((128, 4 * 2048), np.float32)
        ivb = np.zeros((128, 128), np.float32)
        for hl in range(HPC):
            hgl = 4 * g + hl
            wrow = weight[hgl]
            wpad = np.concatenate([np.zeros(127, np.float32), wrow])
            tpc[:, 2048 * hl:2048 * (hl + 1)] = np.lib.stride_tricks.as_strided(
                wpad[127:], shape=(128, S), strides=(-4, 4))
            ivb[:, 16 * hl:16 * (hl + 1)] = invn[hgl].reshape(SB, 128).T
            ivb[:, 64 + 16 * hl:64 + 16 * (hl + 1)] = \
                bias[hgl].reshape(SB, 128).T
        g_pack.append(dict(w1c=w1c, b1x=b1x, w2c=w2c,
                           tpc=tpc.astype(NPBF16), ivb=ivb, ident=ident))

    in_maps = []
    for c in range(N_CORES):
        p, g = c // 4, c % 4
        m = dict(g_pack[g])
        m["xtt"] = xtt_p[p]
        in_maps.append(m)
    return in_maps


def kernel(x, weight, bias, inp_w, inp_b, out_w):
    if "nc" not in _CACHED:
        _CACHED["nc"] = build_program()
    nc = _CACHED["nc"]

    in_maps = host_prep(x, weight, bias, inp_w, inp_b, out_w)
    res = run_bass_kernel_spmd(nc, in_maps, core_ids=list(range(N_CORES)))

    out = np.empty((B, S, E), np.float32)
    for b in range(B):
        p, bb = b // 2, b % 2
        acc = np.zeros((E, S), np.float32)
        for g in range(4):
            acc += np.asarray(res.results[4 * p + g]["outX"][bb],
                              dtype=np.float32)
        out[b] = acc.T
    return out


# revision 16
# speedup vs baseline: 2.8318x; 1.3246x over previous
"""Trainium2 Bass kernel for nn_MultiHeadMixer.

Reference computation (B=4, S=2048, E=1024, H=16, D=64):
    xp = x @ inp_w.T + inp_b                      # (B,S,E)
    xh[b,h,d,s] = xp[b,s,h*D+d]
    y0[b,h,d,t] = sum_{s<=t} xh[b,h,d,s] * weight[h,t-s]   # causal Toeplitz
    y2 = y0 / cumsum(weight)[h,t] + bias[h,t]
    out[b,t,:] = (y2 reshaped to (E,)) @ out_w.T

Sharding (8 cores): core c = (batch-pair p=c//4, head-group g=c%4).
Each core computes batches {2p, 2p+1} for heads {4g..4g+3} (feature cols
F=[256g,256g+256)) and emits a full-(E) bf16 partial of out[b].T per
batch; host sums the four head-group partials per batch and transposes.

On-device phases (PE at full 128 width everywhere):
  proj1:  xp[s, 256b+f] = sum_e xT_b[e,s] w1[e,f]           (K=e)
  mixer:  block-T: y0T[tau,(b,d)] += Tblk[sig,tau]^T xp_i[sig,(b,d)]
          with Tblk = strip[:, 128*(j-i):+128]; both batches of one head
          fill the 128-wide moving tensor; the Delta=j-i loop reuses one
          stationary across a t-quad.
  epi:    y2T = y0T*invn_col + bias_col (per-PARTITION [128,1] scalars)
  transp: PE transposes y2T 64 cols/batch into head-pair PSUM banks.
  proj2:  outT_b[e',t] = sum_c w2[c,e'] y2_b[c,t]           (K=c)

Schedule: the PE stream pipelines one quad deep (transposes/proj2 of
quad q run between mixer segments of q+1) so cross-engine epilogue and
copy latencies never stall the PE.  Loads ride the SP HWDGE ring, output
stores the Activation ring.

PSUM: every bank's first matmul after reuse is start=True (clears the
2KiB zero-region has_written bits); later first-touches overwrite and
repeat-touches accumulate.
"""

import contextlib

import numpy as np
import ml_dtypes

import concourse.bass as bass
import concourse.bacc as bacc
import concourse.mybir as mybir
import concourse.tile as tile
from concourse.bass_utils import run_bass_kernel_spmd

B, S, E, H = 4, 2048, 1024, 16
D = E // H
N_CORES = 8
HPC = 4          # heads per core
FPC = 256        # feature cols per core
SB = S // 128    # 16 s/t-blocks
TQ = S // 512    # 4 t-quads

BF16 = mybir.dt.bfloat16
F32 = mybir.dt.float32
NPBF16 = ml_dtypes.bfloat16

_CACHED = {}

POOL_SPECS = [
    ("xtt", 8, None), ("tpc", 1, None), ("wc", 2, None), ("cst", 3, None),
    ("xp", SB, None), ("y2t", 26, None), ("y2s", 8, None), ("ost", 16, None),
    ("psAP", 2, "PSUM"), ("psm", 4, "PSUM"), ("psT", 2, "PSUM"),
]


def emit_loads(nc, pools, aps, pfx=""):
    xtt_d, w1c_d, b1x_d, tpc_d, ivb_d, w2c_d, ident_d, outX = aps
    t = {}
    t["w1c"] = pools["wc"].tile([128, 2048], BF16, tag="wc", name=f"{pfx}w1c")
    nc.sync.dma_start(t["w1c"][:], w1c_d[:])
    for mg in range(4):
        for b in range(2):
            x = pools["xtt"].tile([128, 4096], BF16, tag="xtt",
                                  name=f"{pfx}xtt_{b}_{mg}")
            eng = nc.sync if b == 0 else nc.scalar
            eng.dma_start(x[:], xtt_d[b][mg])
            t[("xtt", b, mg)] = x
    t["b1"] = pools["cst"].tile([128, 2 * FPC], F32, tag="cst", name=f"{pfx}b1_t")
    nc.sync.dma_start(t["b1"][:], b1x_d[:])
    t["tpc"] = pools["tpc"].tile([128, 4 * 2048], BF16, tag="tpc", name=f"{pfx}tpc")
    nc.scalar.dma_start(t["tpc"][:], tpc_d[:])
    t["ivb"] = pools["cst"].tile([128, 128], F32, tag="cst", name=f"{pfx}ivb")
    nc.sync.dma_start(t["ivb"][:], ivb_d[:])
    t["ident"] = pools["cst"].tile([128, 128], BF16, tag="cst", name=f"{pfx}ident")
    nc.sync.dma_start(t["ident"][:], ident_d[:])
    t["w2c"] = pools["wc"].tile([128, 2048], BF16, tag="wc", name=f"{pfx}w2c")
    nc.scalar.dma_start(t["w2c"][:], w2c_d[:])
    return t


def emit_compute(nc, pools, t, aps, pfx=""):
    xtt_d, w1c_d, b1x_d, tpc_d, ivb_d, w2c_d, ident_d, outX = aps
    AF = mybir.ActivationFunctionType
    ALU = mybir.AluOpType
    w1c, tpc, ivb, ident, w2c = (t["w1c"], t["tpc"], t["ivb"], t["ident"],
                                 t["w2c"])

    def invn_col(h, j):
        return ivb[:, 16 * h + j:16 * h + j + 1]

    def bias_col(h, j):
        return ivb[:, 64 + 16 * h + j:64 + 16 * h + j + 1]

    # ---- proj1 ----
    xp_t = []
    for m in range(SB):
        mg, mm = m // 4, m % 4
        ps = pools["psAP"].tile([128, 512], F32, tag="psAP", name=f"{pfx}ps1_{m}")
        for b in range(2):
            for k in range(8):
                nc.tensor.matmul(
                    ps[:, FPC * b:FPC * (b + 1)],
                    t[("xtt", b, mg)][:, 1024 * mm + 128 * k:
                                      1024 * mm + 128 * (k + 1)],
                    w1c[:, FPC * k:FPC * (k + 1)],
                    start=(b == 0 and k == 0),
                    stop=(b == 1 and k == 7),
                    skip_group_check=True,
                )
        xp = pools["xp"].tile([128, 512], BF16, tag="xp", name=f"{pfx}xp_{m}")
        nc.vector.tensor_add(xp[:], ps[:], t["b1"][:])
        xp_t.append(xp)

    def moving(i, h):
        a = xp_t[i][:].rearrange("p (b h d) -> p b h d", b=2, h=HPC)
        return a[:, :, h, :]

    y2t = {}
    y2s = {}

    def mix(q, hpair, extras=None):
        extras = list(extras or [])
        nmm = 0
        for h in (2 * hpair, 2 * hpair + 1):
            ps = pools["psm"].tile([128, 512], F32, tag="psm",
                                   name=f"{pfx}psm_{h}_{q}")
            for dlt in range(4 * q + 4):
                for j in range(max(4 * q, dlt), 4 * q + 4):
                    i = j - dlt
                    jj = j - 4 * q
                    nc.tensor.matmul(
                        ps[:, 128 * jj:128 * (jj + 1)],
                        tpc[:, 2048 * h + 128 * dlt:
                            2048 * h + 128 * (dlt + 1)],
                        moving(i, h),
                        start=(dlt == 0 and jj == 0),
                        stop=(dlt == 4 * q + 3 and jj == 3),
                        skip_group_check=True,
                    )
                    nmm += 1
                    if extras and nmm % 3 == 0:
                        extras.pop(0)()
            for jj in range(4):
                j = 4 * q + jj
                y = pools["y2t"].tile([128, 128], BF16, tag="y2t",
                                      name=f"{pfx}y2t_{h}_{j}")
                if h % 2 == 0:
                    nc.scalar.activation(
                        y[:], ps[:, 128 * jj:128 * (jj + 1)],
                        AF.Identity,
                        bias=bias_col(h, j), scale=invn_col(h, j))
                else:
                    nc.vector.tensor_scalar(
                        y[:], ps[:, 128 * jj:128 * (jj + 1)],
                        invn_col(h, j), bias_col(h, j),
                        ALU.mult, ALU.add)
                y2t[(h, jj, q)] = y
        for th in extras:
            th()

    def transp_thunks(q, hp):
        """One thunk per transpose (+1 per bank copy) so callers can
        interleave them into a matmul stream, hiding the 64-col
        stationary loads under longer matmuls."""
        thunks = []
        for b in range(2):
            def alloc_and_ops(b=b):
                pst = pools["psT"].tile([128, 1024], BF16, tag="psT",
                                        name=f"{pfx}psT_{hp}_{b}_{q}")
                ops = []
                for hh in range(2):
                    for jj in range(4):
                        ops.append((pst, hh, jj))
                return pst, ops
            pst, ops = alloc_and_ops()
            for pst_, hh, jj in ops:
                def tr(pst_=pst_, hh=hh, jj=jj, b=b):
                    nc.tensor.transpose(
                        pst_[64 * hh:64 * (hh + 1),
                             128 * jj:128 * (jj + 1)],
                        y2t[(2 * hp + hh, jj, q)][:, 64 * b:64 * (b + 1)],
                        ident[:])
                thunks.append(tr)

            def cp(pst_=pst, b=b):
                ys = pools["y2s"].tile([128, 512], BF16, tag="y2s",
                                       name=f"{pfx}y2s_{b}_{hp}_{q}")
                if (hp + b) % 2 == 0:
                    nc.vector.tensor_copy(ys[:], pst_[:, 0:512])
                else:
                    nc.scalar.copy(ys[:], pst_[:, 0:512])
                y2s[(b, hp, q)] = ys
            thunks.append(cp)
        return thunks

    def transp(q, hp):
        for th in transp_thunks(q, hp):
            th()

    ost_t = {}

    def proj2(q, extras=None):
        extras = list(extras or [])
        nmm = 0
        for n in range(8):
            pso = {}
            for hp in range(2):
                for b in range(2):
                    if hp == 0:
                        pso[b] = pools["psAP"].tile(
                            [128, 512], F32, tag="psAP",
                            name=f"{pfx}ps2_{b}_{n}_{q}")
                    nc.tensor.matmul(
                        pso[b][:],
                        w2c[:, 1024 * hp + 128 * n:
                            1024 * hp + 128 * (n + 1)],
                        y2s[(b, hp, q)][:],
                        start=(hp == 0),
                        stop=(hp == 1),
                        skip_group_check=True,
                    )
                    nmm += 1
                    if extras and nmm % 2 == 0:
                        extras.pop(0)()
            for b in range(2):
                if q == 0:
                    ost_t[(b, n)] = pools["ost"].tile(
                        [128, 2048], BF16, tag="ost", name=f"{pfx}ost_{b}_{n}")
                ost = ost_t[(b, n)]
                if n % 2 == 0:
                    nc.vector.tensor_copy(
                        ost[:, 512 * q:512 * (q + 1)], pso[b][:])
                else:
                    nc.scalar.copy(
                        ost[:, 512 * q:512 * (q + 1)], pso[b][:])
                if q == 3:
                    # stores ride the (otherwise idle) GpSimd SWDGE ring
                    nc.gpsimd.dma_start(
                        outX[b][128 * n:128 * (n + 1), :], ost[:])
        for th in extras:
            th()

    # ---- pipelined PE stream: transposes ride inside matmul streams ----
    mix(0, 0)
    mix(0, 1)
    mix(1, 0, extras=transp_thunks(0, 0))
    mix(1, 1, extras=transp_thunks(0, 1))
    proj2(0)
    mix(2, 0, extras=transp_thunks(1, 0))
    mix(2, 1, extras=transp_thunks(1, 1))
    proj2(1)
    mix(3, 0, extras=transp_thunks(2, 0))
    mix(3, 1, extras=transp_thunks(2, 1))
    proj2(2, extras=transp_thunks(3, 0))
    transp(3, 1)
    proj2(3)


def build_program(loop_n=None, mode="full"):
    nc = bacc.Bacc("TRN2", target_bir_lowering=False, debug=False,
                   num_devices=N_CORES)

    aps = (
        nc.dram_tensor("xtt", [2, 4, 128, 4096], BF16,
                       kind="ExternalInput").ap(),
        nc.dram_tensor("w1c", [128, 2048], BF16, kind="ExternalInput").ap(),
        nc.dram_tensor("b1x", [128, 2 * FPC], F32, kind="ExternalInput").ap(),
        nc.dram_tensor("tpc", [128, 4 * 2048], BF16,
                       kind="ExternalInput").ap(),
        nc.dram_tensor("ivb", [128, 128], F32, kind="ExternalInput").ap(),
        nc.dram_tensor("w2c", [128, 2048], BF16, kind="ExternalInput").ap(),
        nc.dram_tensor("ident", [128, 128], BF16, kind="ExternalInput").ap(),
        nc.dram_tensor("outX", [2, E, S], BF16, kind="ExternalOutput").ap(),
    )

    U = 1
    with tile.TileContext(nc) as tc, contextlib.ExitStack() as es:
        pools = {}
        for name, bufs, space in POOL_SPECS:
            kw = dict(space=space) if space else {}
            pools[name] = es.enter_context(
                tc.tile_pool(name=name, bufs=bufs, **kw))

        def body(pfx=""):
            t = emit_loads(nc, pools, aps, pfx)
            emit_compute(nc, pools, t, aps, pfx)

        if mode == "full":
            if not loop_n:
                body()
            else:
                main, rem = divmod(loop_n, U)
                if main:
                    with tc.For_i(0, main, 1, staggered_reset=True):
                        for u in range(U):
                            body(f"u{u}_")
                if rem:
                    with tc.For_i(0, rem, 1, staggered_reset=True):
                        body("r_")
        elif mode == "compute":
            t = emit_loads(nc, pools, aps)
            with (tc.For_i(0, loop_n, 1, staggered_reset=True)
                  if loop_n else contextlib.nullcontext()):
                emit_compute(nc, pools, t, aps)
        elif mode == "dma":
            with (tc.For_i(0, loop_n, 1, staggered_reset=True)
                  if loop_n else contextlib.nullcontext()):
                emit_loads(nc, pools, aps)
        else:
            raise ValueError(mode)

    nc.compile()
    return nc


def host_prep(x, weight, bias, inp_w, inp_b, out_w):
    """Build the 8 per-core input maps (host-side shard + layout prep)."""
    x = np.asarray(x, np.float32)
    weight = np.asarray(weight, np.float32)
    bias = np.asarray(bias, np.float32)
    inp_w = np.asarray(inp_w, np.float32)
    inp_b = np.asarray(inp_b, np.float32)
    out_w = np.asarray(out_w, np.float32)

    invn = 1.0 / np.cumsum(weight, axis=1)
    ident = np.eye(128, dtype=NPBF16)

    # xtt[b, mg][p, (mm,k,s)]: xT-block pretiling so proj1 tile m needs
    # only xtt[:, m//4]
    xtt_p = []
    for p in range(2):
        per_b = []
        for b in (2 * p, 2 * p + 1):
            A = np.ascontiguousarray(x[b].T)                 # [E, S]
            arr = A.reshape(8, 128, 16, 128).transpose(2, 1, 0, 3)  # [m,p,k,s]
            arr = arr.reshape(4, 4, 128, 8, 128).transpose(0, 2, 1, 3, 4)
            per_b.append(arr.reshape(4, 128, 4096))
        xtt_p.append(np.stack(per_b).astype(NPBF16))         # [2,4,128,4096]

    g_pack = []
    for g in range(4):
        cols = slice(FPC * g, FPC * (g + 1))
        w1c = (inp_w[cols, :].T.reshape(8, 128, FPC)
               .transpose(1, 0, 2).reshape(128, 2048)).astype(NPBF16)
        b1row = np.concatenate([inp_b[cols], inp_b[cols]])
        b1x = np.broadcast_to(b1row, (128, 2 * FPC)).astype(np.float32).copy()
        w2c = (out_w[:, cols].T.reshape(2, 128, E)
               .transpose(1, 0, 2).reshape(128, 2048)).astype(NPBF16)
        tpc = np.zeros((128, 4 * 2048), np.float32)
        ivb = np.zeros((128, 128), np.float32)
        for hl in range(HPC):
            hgl = 4 * g + hl
            wrow = weight[hgl]
            wpad = np.concatenate([np.zeros(127, np.float32), wrow])
            tpc[:, 2048 * hl:2048 * (hl + 1)] = np.lib.stride_tricks.as_strided(
                wpad[127:], shape=(128, S), strides=(-4, 4))
            ivb[:, 16 * hl:16 * (hl + 1)] = invn[hgl].reshape(SB, 128).T
            ivb[:, 64 + 16 * hl:64 + 16 * (hl + 1)] = \
                bias[hgl].reshape(SB, 128).T
        g_pack.append(dict(w1c=w1c, b1x=b1x, w2c=w2c,
                           tpc=tpc.astype(NPBF16), ivb=ivb, ident=ident))

    in_maps = []
    for c in range(N_CORES):
        p, g = c // 4, c % 4
        m = dict(g_pack[g])
        m["xtt"] = xtt_p[p]
        in_maps.append(m)
    return in_maps


def kernel(x, weight, bias, inp_w, inp_b, out_w):
    if "nc" not in _CACHED:
        _CACHED["nc"] = build_program()
    nc = _CACHED["nc"]

    in_maps = host_prep(x, weight, bias, inp_w, inp_b, out_w)
    res = run_bass_kernel_spmd(nc, in_maps, core_ids=list(range(N_CORES)))

    out = np.empty((B, S, E), np.float32)
    for b in range(B):
        p, bb = b // 2, b % 2
        acc = np.zeros((E, S), np.float32)
        for g in range(4):
            acc += np.asarray(res.results[4 * p + g]["outX"][bb],
                              dtype=np.float32)
        out[b] = acc.T
    return out
